# revision 11
# baseline (speedup 1.0000x reference)
"""Self-contained Trainium2 kernel for nn_AdaptiveMoEWithSkip.

Strategy:
  - Host: gate forward (numpy, exact math), top-1 routing, sample dispatch,
    BN-folding + weight prepacking into lhsT layouts (bf16), input padding.
  - Device (8 NeuronCores, SPMD): each core runs one expert's ResNet on up to
    CCAP dispatched samples + the skip net on its 64 original-order samples.
    Outputs partial logits for both paths; host gathers and adds.
"""
import contextlib

import numpy as np
import ml_dtypes

import concourse.bass as bass
import concourse.bacc as bacc
import concourse.mybir as mybir
import concourse.tile as tile
from concourse import bass_utils

F32 = mybir.dt.float32
BF = mybir.dt.bfloat16
AF = mybir.ActivationFunctionType
ALU = mybir.AluOpType
BF_NP = ml_dtypes.bfloat16

E = 4
B = 512
N_CORES = 8
EPS = 1e-5
NBIAS = 40  # bias table columns

# ---------------------------------------------------------------------------
# Host-side gate (numpy, f32)
# ---------------------------------------------------------------------------


def _im2col(x, kh, kw, stride=1, pad=1):
    # x [N, C, H, W] -> patches [N, Ho, Wo, C, kh, kw]
    N, C, H, W = x.shape
    xp = np.pad(x, ((0, 0), (0, 0), (pad, pad), (pad, pad)))
    Ho = (H + 2 * pad - kh) // stride + 1
    Wo = (W + 2 * pad - kw) // stride + 1
    sN, sC, sH, sW = xp.strides
    view = np.lib.stride_tricks.as_strided(
        xp, (N, Ho, Wo, C, kh, kw),
        (sN, sH * stride, sW * stride, sC, sH, sW), writeable=False)
    return view, Ho, Wo


def _gate_top1(x, gp):
    s_bn = np.float32(1.0 / np.sqrt(1.0 + EPS))
    w1 = np.asarray(gp['c1'], np.float32)     # [16,3,3,3]
    patches, Ho, Wo = _im2col(x, 3, 3)
    h = patches.reshape(B * Ho * Wo, 27) @ w1.reshape(16, 27).T
    h = h + np.asarray(gp['c1b'], np.float32)
    h = h * (np.asarray(gp['g1'], np.float32) * s_bn) + np.asarray(gp['b1'], np.float32)
    h = np.maximum(h, 0).reshape(B, 32, 32, 16)
    # maxpool 2x2 stride 2
    h = h.reshape(B, 16, 2, 16, 2, 16).max(axis=(2, 4))  # [B,16,16,16] (y,x,c)
    h = h.transpose(0, 3, 1, 2)  # [B,16,16,16] NCHW
    w2 = np.asarray(gp['c2'], np.float32)     # [32,16,3,3]
    patches, Ho, Wo = _im2col(h, 3, 3)
    h = patches.reshape(B * Ho * Wo, 144) @ w2.reshape(32, 144).T
    h = h + np.asarray(gp['c2b'], np.float32)
    h = h * (np.asarray(gp['g2'], np.float32) * s_bn) + np.asarray(gp['b2'], np.float32)
    h = np.maximum(h, 0).reshape(B, 16 * 16, 32).mean(axis=1)  # GAP [B,32]
    h = np.maximum(h @ np.asarray(gp['fc1'], np.float32).T
                   + np.asarray(gp['fc1b'], np.float32), 0)
    logits = h @ np.asarray(gp['fc2'], np.float32).T + np.asarray(gp['fc2b'], np.float32)
    return logits.argmax(axis=1)


# ---------------------------------------------------------------------------
# Host-side weight packing
# ---------------------------------------------------------------------------


def _fold(w, g):
    s = np.asarray(g, np.float32) * np.float32(1.0 / np.sqrt(1.0 + EPS))
    return np.asarray(w, np.float32) * s[:, None, None, None]


def _lhsT_taps(wf, repl, rows_per_group=None):
    """wf [co, ci, kh, kw] -> [128, kh*kw*co] bf16; rows g*stride + ci."""
    co, ci, kh, kw = wf.shape
    a = wf.transpose(1, 2, 3, 0).reshape(ci, kh * kw * co)  # [ci, t*co]
    stride = rows_per_group or ci
    out = np.zeros((128, kh * kw * co), np.float32)
    for g in range(repl):
        out[g * stride:g * stride + ci] = a
    return out.astype(BF_NP)


def _lhsT_stem(wf):
    """wf [32,3,3,3] -> [128, 32]; rows g*32 + (t*3+ci), t=dy*3+dx."""
    co = wf.shape[0]
    a = wf.transpose(2, 3, 1, 0).reshape(9 * 3, co)  # [(dy,dx,ci), co]
    out = np.zeros((128, co), np.float32)
    for g in range(4):
        out[g * 32:g * 32 + 27] = a
    return out.astype(BF_NP)


def _lhsT_skip_stem(wf):
    """wf [64,3,7,7] -> [128, 7*64]; rows g*64 + (dy*3+ci), cols dx*64+co."""
    a = wf.transpose(2, 1, 3, 0)  # [dy, ci, dx, co]
    a = a.reshape(7 * 3, 7 * 64)  # [(dy,ci), (dx,co)]
    out = np.zeros((128, 7 * 64), np.float32)
    for g in range(2):
        out[g * 64:g * 64 + 21] = a
    return out.astype(BF_NP)


def _lhsT_mh(wf, n_kt, n_mh):
    """wf [co_total, ci_total, kh, kw] -> [128, (mh,kt,t)*128] bf16."""
    co_t, ci_t, kh, kw = wf.shape
    taps = kh * kw
    cols = []
    for mh in range(n_mh):
        for kt in range(n_kt):
            blk = wf[mh * 128:(mh + 1) * 128, kt * 128:(kt + 1) * 128]
            cols.append(blk.transpose(1, 2, 3, 0).reshape(128, taps * 128))
    return np.concatenate(cols, axis=1).astype(BF_NP)


class _BiasTab:
    def __init__(self):
        self.tab = np.zeros((128, NBIAS), np.float32)
        self.n = 0

    def add(self, b, repl, stride=None):
        b = np.asarray(b, np.float32)
        stride = stride or len(b)
        col = self.n
        for g in range(repl):
            self.tab[g * stride:g * stride + len(b), col] = b
        self.n += 1
        return col


def pack_expert(ep, e, skip_params, final_W, final_b):
    """Pack weights for expert e + skip net. Returns (tensors dict, bias cols)."""
    t = {}
    bt = _BiasTab()
    cols = {}

    def grab(d, k):
        return np.asarray(d[k], np.float32)[e]

    w = _fold(grab(ep, 'conv1'), grab(ep, 'bn1_g'))
    t['w_stem'] = _lhsT_stem(w)
    cols['stem'] = bt.add(grab(ep, 'bn1_b'), 4)

    # L0: two blocks, 32ch, 4-group diag (rows g*32+ci)
    for b in range(2):
        blk = {k: np.asarray(v, np.float32)[e] for k, v in ep[f'l0b{b}'].items()}
        t[f'w_l0b{b}c1'] = _lhsT_taps(_fold(blk['c1'], blk['g1']), 4, 32)
        cols[f'l0b{b}c1'] = bt.add(blk['b1'], 4)
        t[f'w_l0b{b}c2'] = _lhsT_taps(_fold(blk['c2'], blk['g2']), 4, 32)
        cols[f'l0b{b}c2'] = bt.add(blk['b2'], 4)

    # L1: 64ch
    blk = {k: np.asarray(v, np.float32)[e] for k, v in ep['l1b0'].items()}
    t['w_l1b0c1'] = _lhsT_taps(_fold(blk['c1'], blk['g1']), 4, 32)   # K=32 rows r*32
    cols['l1b0c1'] = bt.add(blk['b1'], 2)
    t['w_l1b0c2'] = _lhsT_taps(_fold(blk['c2'], blk['g2']), 2, 64)
    cols['l1b0c2'] = bt.add(blk['b2'], 2)
    t['w_l1b0sc'] = _lhsT_taps(_fold(blk['sc'], blk['scg']), 4, 32)  # 1x1
    cols['l1b0sc'] = bt.add(blk['scb'], 2)
    blk = {k: np.asarray(v, np.float32)[e] for k, v in ep['l1b1'].items()}
    t['w_l1b1c1'] = _lhsT_taps(_fold(blk['c1'], blk['g1']), 2, 64)
    cols['l1b1c1'] = bt.add(blk['b1'], 2)
    t['w_l1b1c2'] = _lhsT_taps(_fold(blk['c2'], blk['g2']), 2, 64)
    cols['l1b1c2'] = bt.add(blk['b2'], 2)

    # L2: 128ch
    blk = {k: np.asarray(v, np.float32)[e] for k, v in ep['l2b0'].items()}
    t['w_l2b0c1'] = _lhsT_taps(_fold(blk['c1'], blk['g1']), 2, 64)   # K=64 rows g*64
    cols['l2b0c1'] = bt.add(blk['b1'], 1)
    t['w_l2b0c2'] = _lhsT_taps(_fold(blk['c2'], blk['g2']), 1, 128)
    cols['l2b0c2'] = bt.add(blk['b2'], 1)
    t['w_l2b0sc'] = _lhsT_taps(_fold(blk['sc'], blk['scg']), 2, 64)
    cols['l2b0sc'] = bt.add(blk['scb'], 1)
    blk = {k: np.asarray(v, np.float32)[e] for k, v in ep['l2b1'].items()}
    t['w_l2b1c1'] = _lhsT_taps(_fold(blk['c1'], blk['g1']), 1, 128)
    cols['l2b1c1'] = bt.add(blk['b1'], 1)
    t['w_l2b1c2'] = _lhsT_taps(_fold(blk['c2'], blk['g2']), 1, 128)
    cols['l2b1c2'] = bt.add(blk['b2'], 1)

    # L3: 256ch, M-halves
    blk = {k: np.asarray(v, np.float32)[e] for k, v in ep['l3b0'].items()}
    t['w_l3b0c1'] = _lhsT_mh(_fold(blk['c1'], blk['g1']), 1, 2)
    cols['l3b0c1_h0'] = bt.add(blk['b1'][:128], 1)
    cols['l3b0c1_h1'] = bt.add(blk['b1'][128:], 1)
    t['w_l3b0c2'] = _lhsT_mh(_fold(blk['c2'], blk['g2']), 2, 2)
    cols['l3b0c2_h0'] = bt.add(blk['b2'][:128], 1)
    cols['l3b0c2_h1'] = bt.add(blk['b2'][128:], 1)
    t['w_l3b0sc'] = _lhsT_mh(_fold(blk['sc'], blk['scg']), 1, 2)
    cols['l3b0sc_h0'] = bt.add(blk['scb'][:128], 1)
    cols['l3b0sc_h1'] = bt.add(blk['scb'][128:], 1)
    blk = {k: np.asarray(v, np.float32)[e] for k, v in ep['l3b1'].items()}
    t['w_l3b1c1'] = _lhsT_mh(_fold(blk['c1'], blk['g1']), 2, 2)
    cols['l3b1c1_h0'] = bt.add(blk['b1'][:128], 1)
    cols['l3b1c1_h1'] = bt.add(blk['b1'][128:], 1)
    t['w_l3b1c2'] = _lhsT_mh(_fold(blk['c2'], blk['g2']), 2, 2)
    cols['l3b1c2_h0'] = bt.add(blk['b2'][:128], 1)
    cols['l3b1c2_h1'] = bt.add(blk['b2'][128:], 1)

    # expert head: W1 = final_W[:, :256], fold avgpool 1/16
    fw = np.asarray(final_W, np.float32)
    w1 = fw[:, :256] / 16.0  # [10, 256]
    t['w_head'] = np.concatenate(
        [w1[:, :128].T, w1[:, 128:].T], axis=1).astype(BF_NP)  # [128, 20]

    # skip net
    sp = {k: np.asarray(v, np.float32) for k, v in skip_params.items()}
    t['ws_stem'] = _lhsT_skip_stem(_fold(sp['c1'], sp['g1']))
    cols['s_stem'] = bt.add(sp['c1b'] * (sp['g1'] / np.sqrt(1 + EPS)) + sp['b1'], 2)
    t['ws_c2'] = _lhsT_taps(_fold(sp['c2'], sp['g2']), 2, 64)
    cols['s_c2'] = bt.add(sp['c2b'] * (sp['g2'] / np.sqrt(1 + EPS)) + sp['b2'], 1)
    t['ws_c3'] = _lhsT_mh(_fold(sp['c3'], sp['g3']), 1, 2)
    b3 = sp['c3b'] * (sp['g3'] / np.sqrt(1 + EPS)) + sp['b3']
    cols['s_c3_h0'] = bt.add(b3[:128], 1)
    cols['s_c3_h1'] = bt.add(b3[128:], 1)
    # skip head: contrib = hm @ (W2 @ fc).T + (W2 @ fcb + final_b); fold 1/4 avgpool
    w2 = fw[:, 256:]  # [10, 256]
    m = (w2 @ sp['fc']) / 4.0  # [10, 256]
    t['ws_head'] = np.concatenate(
        [m[:, :128].T, m[:, 128:].T], axis=1).astype(BF_NP)  # [128, 20]
    cols['s_head'] = bt.add(w2 @ sp['fcb'] + np.asarray(final_b, np.float32), 1)

    assert bt.n <= NBIAS
    t['bias'] = bt.tab
    return t, cols


# ---------------------------------------------------------------------------
# Device program
# ---------------------------------------------------------------------------


def build_program(ccap, loop_n=1):
    nc = bacc.Bacc("TRN2", target_bir_lowering=False, debug=False,
                   num_devices=N_CORES)
    CC = ccap

    xe = nc.dram_tensor("xe", [CC, 3, 36, 36], BF, kind="ExternalInput")
    xs = nc.dram_tensor("xs", [64, 3, 38, 38], BF, kind="ExternalInput")
    wshapes = {
        'w_stem': 32, 'w_l0b0c1': 288, 'w_l0b0c2': 288, 'w_l0b1c1': 288,
        'w_l0b1c2': 288, 'w_l1b0c1': 576, 'w_l1b0c2': 576, 'w_l1b0sc': 64,
        'w_l1b1c1': 576, 'w_l1b1c2': 576, 'w_l2b0c1': 1152, 'w_l2b0c2': 1152,
        'w_l2b0sc': 128, 'w_l2b1c1': 1152, 'w_l2b1c2': 1152,
        'w_l3b0c1': 2304, 'w_l3b0c2': 4608, 'w_l3b0sc': 256,
        'w_l3b1c1': 4608, 'w_l3b1c2': 4608, 'w_head': 20,
        'ws_stem': 448, 'ws_c2': 1152, 'ws_c3': 2304, 'ws_head': 20,
    }
    WD = {k: nc.dram_tensor(k, [128, v], BF, kind="ExternalInput")
          for k, v in wshapes.items()}
    bias_d = nc.dram_tensor("bias", [128, NBIAS], F32, kind="ExternalInput")
    oe = nc.dram_tensor("oe", [10, CC], F32, kind="ExternalOutput")
    os_ = nc.dram_tensor("os", [10, 64], F32, kind="ExternalOutput")

    SG = CC // 4   # samples per stem/L0 group (16 when CC=64)
    SG2 = CC // 2  # samples per L1-style group

    with tile.TileContext(nc) as tc:
        with contextlib.ExitStack() as ctx:
            act = ctx.enter_context(tc.tile_pool(name="act", bufs=4))
            wp = ctx.enter_context(tc.tile_pool(name="wp", bufs=3))
            psp = ctx.enter_context(tc.tile_pool(name="ps", bufs=6, space="PSUM"))
            cp = ctx.enter_context(tc.tile_pool(name="cp", bufs=1))
            sp = ctx.enter_context(tc.tile_pool(name="sp", bufs=2))

            tbias = cp.tile([128, NBIAS], F32)
            nc.sync.dma_start(tbias[:], bias_d[:])

            def bias_ap(col, p0=0, pn=128):
                return tbias[p0:p0 + pn, col:col + 1]

            def wload(name):
                wt = wp.tile([128, wshapes[name]], BF, tag="w")
                nc.sync.dma_start(wt[:], WD[name][:])
                return wt

            def new_act(ns, F):
                """Padded act tile [128, ns*F*F] with zeroed 1-ring border."""
                t = act.tile([128, ns * F * F], BF, tag="act")
                v = t[:].rearrange("p (s y x) -> p s y x", y=F, x=F)
                nc.gpsimd.memset(v[:, :, 0:1, :], 0.0)
                nc.gpsimd.memset(v[:, :, F - 1:F, :], 0.0)
                nc.gpsimd.memset(v[:, :, 1:F - 1, 0:1], 0.0)
                nc.gpsimd.memset(v[:, :, 1:F - 1, F - 1:F], 0.0)
                return t, v

            def new_tight(ns, h, w_):
                t = act.tile([128, ns * h * w_], BF, tag="act")
                return t, t[:].rearrange("p (s y x) -> p s y x", y=h, x=w_)

            # ---------------- generic conv executor ----------------
            def run_conv(units, n_ps, chunks, rhs_fn, lhsT_fn, evacs):
                """units: list of (ps_idx, ps_p0, M, u) — one matmul stream each.
                chunks: list of chunk descriptors (opaque to us).
                rhs_fn(u, ch, k) -> (rhs_ap, tile_pos); k iterates taps/kt.
                lhsT_fn(u, k) -> lhsT ap.  nk = len of k-list per unit.
                evacs: list of (ps_idx, fn(ps_tile, ch)) run per chunk."""
                for ch in chunks:
                    pst = [psp.tile([128, 512], F32, tag="ps", name=f"ps{i}")
                           for i in range(n_ps)]
                    for (ps_idx, ps_p0, M, u) in units:
                        ks = rhs_fn(u, ch, None)  # list of k keys
                        for ki, k in enumerate(ks):
                            rhs, tp = rhs_fn(u, ch, k)
                            n = rhs.free_size()
                            nc.tensor.matmul(
                                pst[ps_idx][ps_p0:ps_p0 + M, 0:n],
                                lhsT_fn(u, k), rhs,
                                start=(ki == 0), stop=(ki == len(ks) - 1),
                                tile_position=tp)
                    for (ps_idx, fn) in evacs:
                        fn(pst[ps_idx], ch)

            # ================= expert network =================
            def expert_net():
                # ---- stem: K=27 im2col patches, 4-group diag ----
                # layout: P[g*32 + t*3+ci, (s, i, j)]
                pt, pv4 = new_tight(SG, 32, 32)
                xet = xe[:].rearrange("s c y x -> c s y x")
                for g in range(4):
                    for t9 in range(9):
                        dy, dx = t9 // 3, t9 % 3
                        for ci in range(3):
                            nc.sync.dma_start(
                                pv4[g * 32 + t9 * 3 + ci:g * 32 + t9 * 3 + ci + 1,
                                    0:SG, :, :],
                                xet[ci:ci + 1, g * SG:(g + 1) * SG,
                                    1 + dy:33 + dy, 1 + dx:33 + dx])
                w_st = wload('w_stem')
                a_st, a_stv = new_act(SG, 34)

                def stem_rhs(u, ch, k):
                    if k is None:
                        return [0]
                    g = u
                    s, ih = ch
                    return (pv4[g * 32:g * 32 + 27, s:s + 1,
                                ih * 16:(ih + 1) * 16, :], (32 * g, 32 * g))

                def stem_evac(ps, ch):
                    s, ih = ch
                    nc.scalar.activation(
                        a_stv[:, s:s + 1, 1 + ih * 16:1 + (ih + 1) * 16, 1:33],
                        ps[:, 0:512], AF.Relu, bias=bias_ap(COLS['stem']), scale=1.0)

                run_conv([(0, 32 * g, 32, g) for g in range(4)], 1,
                         [(s, ih) for s in range(SG) for ih in range(2)],
                         stem_rhs, lambda u, k: w_st[u * 32:u * 32 + 27, :],
                         [(0, stem_evac)])

                # ---- L0 blocks: 4-group diag, K=32, M=32 ----
                def conv_l0(in_v, out_v, wname, bcol, resid_v=None, relu=True):
                    wt = wload(wname)

                    def rhs(u, ch, k):
                        if k is None:
                            return list(range(9))
                        g = u
                        s, ih = ch
                        dy, dx = k // 3, k % 3
                        return (in_v[g * 32:(g + 1) * 32, s:s + 1,
                                     ih * 16 + dy:ih * 16 + dy + 16, dx:dx + 32],
                                (32 * g, 32 * g))

                    def evac(ps, ch):
                        s, ih = ch
                        oap = out_v[:, s:s + 1, 1 + ih * 16:1 + (ih + 1) * 16, 1:33]
                        if resid_v is None:
                            nc.scalar.activation(oap, ps[:, 0:512], AF.Relu,
                                                 bias=bias_ap(bcol), scale=1.0)
                        else:
                            rap = resid_v[:, s:s + 1,
                                          1 + ih * 16:1 + (ih + 1) * 16, 1:33]
                            nc.vector.tensor_add(ps[:, 0:512], ps[:, 0:512],
                                                 rap)
                            nc.scalar.activation(oap, ps[:, 0:512], AF.Relu,
                                                 bias=bias_ap(bcol), scale=1.0)

                    run_conv([(0, 32 * g, 32, g) for g in range(4)], 1,
                             [(s, ih) for s in range(SG) for ih in range(2)],
                             rhs, lambda u, k: wt[u * 32:(u + 1) * 32,
                                                  k * 32:(k + 1) * 32],
                             [(0, evac)])

                y1, y1v = new_act(SG, 34)
                conv_l0(a_stv, y1v, 'w_l0b0c1', COLS['l0b0c1'])
                y2, y2v = new_act(SG, 34)
                conv_l0(y1v, y2v, 'w_l0b0c2', COLS['l0b0c2'], resid_v=a_stv)
                y3, y3v = new_act(SG, 34)
                conv_l0(y2v, y3v, 'w_l0b1c1', COLS['l0b1c1'])
                y4, y4v = new_act(SG, 34)
                conv_l0(y3v, y4v, 'w_l0b1c2', COLS['l0b1c2'], resid_v=y2v)

                # ---- L1 b0 c1: K=32 4 row-tiles -> 2 col-groups, stride 2 ----
                # psum p: row-tiles {p (smp slots 0:16 of both out-groups? see below)}
                l1a, l1av = new_act(SG2, 18)
                wt = wload('w_l1b0c1')

                def l1c1_rhs(u, ch, k):
                    if k is None:
                        return list(range(9))
                    g0 = u  # L0 group index 0..3
                    s, ih = ch  # 4-sample block, row half
                    dy, dx = k // 3, k % 3
                    # out 16x16, stride2: in rows 2i+dy
                    return (y4v[g0 * 32:(g0 + 1) * 32, 4 * s:4 * s + 4,
                                ih * 16 + dy:ih * 16 + dy + 16:2, dx:dx + 32:2],
                            (32 * g0, 64 * (g0 // 2)))

                def l1c1_evac_mk(ps_idx):
                    def evac(ps, ch):
                        s, ih = ch
                        # psum ps_idx covers row-tiles g0 in {ps_idx, ps_idx+2}:
                        # within-group smp slots [ps_idx*16, ps_idx*16+16)
                        slot = ps_idx * SG
                        oap = l1av[:, 4 * s + slot:4 * s + slot + 4,
                                   1 + ih * 8:1 + (ih + 1) * 8, 1:17]
                        nc.scalar.activation(oap, ps[:, 0:512], AF.Relu,
                                             bias=bias_ap(COLS['l1b0c1']),
                                             scale=1.0)
                    return evac

                # units: (ps_idx = g0 % 2, ps_p0 = 64*(g0//2), M=64, u=g0)
                run_conv([(g0 % 2, 64 * (g0 // 2), 64, g0) for g0 in range(4)], 2,
                         [(s, ih) for s in range(SG // 4) for ih in range(2)],
                         l1c1_rhs, lambda u, k: wt[u * 32:(u + 1) * 32,
                                                   k * 64:(k + 1) * 64],
                         [(0, l1c1_evac_mk(0)), (1, l1c1_evac_mk(1))])

                # ---- L1 b0 sc: 1x1 stride2, same structure, no relu ----
                sc1, sc1v = new_act(SG2, 18)
                wt = wload('w_l1b0sc')

                def l1sc_rhs(u, ch, k):
                    if k is None:
                        return [0]
                    g0 = u
                    s, ih = ch
                    return (y4v[g0 * 32:(g0 + 1) * 32, 4 * s:4 * s + 4,
                                ih * 16 + 1:ih * 16 + 17:2, 1:33:2],
                            (32 * g0, 64 * (g0 // 2)))

                def l1sc_evac_mk(ps_idx):
                    def evac(ps, ch):
                        s, ih = ch
                        slot = ps_idx * SG
                        oap = sc1v[:, 4 * s + slot:4 * s + slot + 4,
                                   1 + ih * 8:1 + (ih + 1) * 8, 1:17]
                        nc.scalar.activation(oap, ps[:, 0:512], AF.Identity,
                                             bias=bias_ap(COLS['l1b0sc']),
                                             scale=1.0)
                    return evac

                run_conv([(g0 % 2, 64 * (g0 // 2), 64, g0) for g0 in range(4)], 2,
                         [(s, ih) for s in range(SG // 4) for ih in range(2)],
                         l1sc_rhs, lambda u, k: wt[u * 32:(u + 1) * 32, 0:64],
                         [(0, l1sc_evac_mk(0)), (1, l1sc_evac_mk(1))])

                # ---- L1 64ch convs: 2-group diag K=64 M=64 ----
                def conv_l1(in_v, out_v, wname, bcol, resid_v=None):
                    wt = wload(wname)

                    def rhs(u, ch, k):
                        if k is None:
                            return list(range(9))
                        g = u
                        s, ih = ch
                        dy, dx = k // 3, k % 3
                        return (in_v[g * 64:(g + 1) * 64, 4 * s:4 * s + 4,
                                     ih * 8 + dy:ih * 8 + dy + 8, dx:dx + 16],
                                (64 * g, 64 * g))

                    def evac(ps, ch):
                        s, ih = ch
                        oap = out_v[:, 4 * s:4 * s + 4,
                                    1 + ih * 8:1 + (ih + 1) * 8, 1:17]
                        if resid_v is None:
                            nc.scalar.activation(oap, ps[:, 0:512], AF.Relu,
                                                 bias=bias_ap(bcol), scale=1.0)
                        else:
                            rap = resid_v[:, 4 * s:4 * s + 4,
                                          1 + ih * 8:1 + (ih + 1) * 8, 1:17]
                            nc.vector.tensor_add(ps[:, 0:512], ps[:, 0:512],
                                                 rap)
                            nc.scalar.activation(oap, ps[:, 0:512], AF.Relu,
                                                 bias=bias_ap(bcol), scale=1.0)

                    run_conv([(0, 64 * g, 64, g) for g in range(2)], 1,
                             [(s, ih) for s in range(SG2 // 4) for ih in range(2)],
                             rhs, lambda u, k: wt[u * 64:(u + 1) * 64,
                                                  k * 64:(k + 1) * 64],
                             [(0, evac)])

                l1b, l1bv = new_act(SG2, 18)
                conv_l1(l1av, l1bv, 'w_l1b0c2', COLS['l1b0c2'], resid_v=sc1v)
                l1c, l1cv = new_act(SG2, 18)
                conv_l1(l1bv, l1cv, 'w_l1b1c1', COLS['l1b1c1'])
                l1d, l1dv = new_act(SG2, 18)
                conv_l1(l1cv, l1dv, 'w_l1b1c2', COLS['l1b1c2'], resid_v=l1bv)

                # ---- L2 b0 c1 (K=64 2 row-tiles, M=128, stride 2) ----
                l2a, l2av = new_act(CC, 10)
                wt = wload('w_l2b0c1')

                def l2c1_rhs(u, ch, k):
                    if k is None:
                        return list(range(9))
                    g = u
                    (sb,) = ch  # 8-sample block within group g
                    dy, dx = k // 3, k % 3
                    return (l1dv[g * 64:(g + 1) * 64, sb * 8:sb * 8 + 8,
                                 dy:dy + 16:2, dx:dx + 16:2],
                            (64 * g, 0))

                def l2c1_evac_mk(g):
                    def evac(ps, ch):
                        (sb,) = ch
                        oap = l2av[:, g * SG2 + sb * 8:g * SG2 + sb * 8 + 8,
                                   1:9, 1:9]
                        nc.scalar.activation(oap, ps[:, 0:512], AF.Relu,
                                             bias=bias_ap(COLS['l2b0c1']),
                                             scale=1.0)
                    return evac

                run_conv([(g, 0, 128, g) for g in range(2)], 2,
                         [(sb,) for sb in range(SG2 // 8)],
                         l2c1_rhs, lambda u, k: wt[u * 64:(u + 1) * 64,
                                                   k * 128:(k + 1) * 128],
                         [(0, l2c1_evac_mk(0)), (1, l2c1_evac_mk(1))])

                # ---- L2 b0 sc (K=64 2 row-tiles, 1x1 stride2) ----
                sc2, sc2v = new_act(CC, 10)
                wt = wload('w_l2b0sc')

                def l2sc_rhs(u, ch, k):
                    if k is None:
                        return [0]
                    g = u
                    (sb,) = ch
                    return (l1dv[g * 64:(g + 1) * 64, sb * 8:sb * 8 + 8,
                                 1:17:2, 1:17:2], (64 * g, 0))

                def l2sc_evac_mk(g):
                    def evac(ps, ch):
                        (sb,) = ch
                        oap = sc2v[:, g * SG2 + sb * 8:g * SG2 + sb * 8 + 8,
                                   1:9, 1:9]
                        nc.scalar.activation(oap, ps[:, 0:512], AF.Identity,
                                             bias=bias_ap(COLS['l2b0sc']),
                                             scale=1.0)
                    return evac

                run_conv([(g, 0, 128, g) for g in range(2)], 2,
                         [(sb,) for sb in range(SG2 // 8)],
                         l2sc_rhs, lambda u, k: wt[u * 64:(u + 1) * 64, 0:128],
                         [(0, l2sc_evac_mk(0)), (1, l2sc_evac_mk(1))])

                # ---- L2 128ch convs (K=128, M=128, single tile) ----
                def conv_l2(in_v, out_v, wname, bcol, resid_v=None):
                    wt = wload(wname)

                    def rhs(u, ch, k):
                        if k is None:
                            return list(range(9))
                        (sb,) = ch
                        dy, dx = k // 3, k % 3
                        return (in_v[:, sb * 8:sb * 8 + 8, dy:dy + 8, dx:dx + 8],
                                None)

                    def evac(ps, ch):
                        (sb,) = ch
                        oap = out_v[:, sb * 8:sb * 8 + 8, 1:9, 1:9]
                        if resid_v is None:
                            nc.scalar.activation(oap, ps[:, 0:512], AF.Relu,
                                                 bias=bias_ap(bcol), scale=1.0)
                        else:
                            rap = resid_v[:, sb * 8:sb * 8 + 8, 1:9, 1:9]
                            nc.vector.tensor_add(ps[:, 0:512], ps[:, 0:512],
                                                 rap)
                            nc.scalar.activation(oap, ps[:, 0:512], AF.Relu,
                                                 bias=bias_ap(bcol), scale=1.0)

                    run_conv([(0, 0, 128, 0)], 1,
                             [(sb,) for sb in range(CC // 8)],
                             rhs, lambda u, k: wt[:, k * 128:(k + 1) * 128],
                             [(0, evac)])

                l2b, l2bv = new_act(CC, 10)
                conv_l2(l2av, l2bv, 'w_l2b0c2', COLS['l2b0c2'], resid_v=sc2v)
                l2c, l2cv = new_act(CC, 10)
                conv_l2(l2bv, l2cv, 'w_l2b1c1', COLS['l2b1c1'])
                l2d, l2dv = new_act(CC, 10)
                conv_l2(l2cv, l2dv, 'w_l2b1c2', COLS['l2b1c2'], resid_v=l2bv)

                # ---- L3: 256ch as [128, (kh, s, 6, 6)] ----
                def l3_view(t):
                    return t[:].rearrange("p (h s y x) -> p h s y x", h=2, y=6, x=6)

                def conv_l3(in_v, out_v, wname, bcols, n_kt, stride, in_is_l2,
                            resid_v=None, taps=9):
                    """in_v: l2-style [p, s, 10, 10] if in_is_l2 else l3 view."""
                    wt = wload(wname)
                    nm = CC // 32  # chunks of 32 samples

                    def rhs_mk(mh):
                        def rhs(u, ch, k):
                            if k is None:
                                return [(kt, t) for kt in range(n_kt)
                                        for t in range(taps)]
                            (sb,) = ch
                            kt, t = k
                            dy, dx = t // 3, t % 3
                            if taps == 1:
                                dy, dx = 1, 1  # 1x1 conv reads center
                            if in_is_l2:
                                return (in_v[:, sb * 32:sb * 32 + 32,
                                             dy:dy + 8:2, dx:dx + 8:2], None)
                            return (in_v[:, kt, sb * 32:sb * 32 + 32,
                                         dy:dy + 4, dx:dx + 4], None)
                        return rhs

                    def lhsT_mk(mh):
                        def lf(u, k):
                            kt, t = k
                            col = ((mh * n_kt + kt) * taps + t) * 128
                            return wt[:, col:col + 128]
                        return lf

                    def evac_mk(mh):
                        def evac(ps, ch):
                            (sb,) = ch
                            oap = out_v[:, mh, sb * 32:sb * 32 + 32, 1:5, 1:5]
                            bc = bcols[mh]
                            if resid_v is None:
                                nc.scalar.activation(oap, ps[:, 0:512], AF.Relu,
                                                     bias=bias_ap(bc), scale=1.0)
                            else:
                                rap = resid_v[:, mh, sb * 32:sb * 32 + 32,
                                              1:5, 1:5]
                                nc.vector.tensor_add(ps[:, 0:512], ps[:, 0:512],
                                                     rap)
                                nc.scalar.activation(oap, ps[:, 0:512], AF.Relu,
                                                     bias=bias_ap(bc), scale=1.0)
                        return evac

                    for mh in range(2):
                        run_conv([(0, 0, 128, 0)], 1,
                                 [(sb,) for sb in range(nm)],
                                 rhs_mk(mh), lhsT_mk(mh), [(0, evac_mk(mh))])

                l3a = act.tile([128, 2 * CC * 36], BF, tag="act")
                l3av = l3_view(l3a)
                for mh in range(2):
                    v = l3av[:, mh]
                    nc.gpsimd.memset(v[:, :, 0:1, :], 0.0)
                    nc.gpsimd.memset(v[:, :, 5:6, :], 0.0)
                    nc.gpsimd.memset(v[:, :, 1:5, 0:1], 0.0)
                    nc.gpsimd.memset(v[:, :, 1:5, 5:6], 0.0)
                conv_l3(l2dv, l3av, 'w_l3b0c1',
                        [COLS['l3b0c1_h0'], COLS['l3b0c1_h1']], 1, 2, True)

                sc3 = act.tile([128, 2 * CC * 36], BF, tag="act")
                sc3v = l3_view(sc3)

                def l3sc():
                    wt = wload('w_l3b0sc')
                    for mh in range(2):
                        for sb in range(CC // 32):
                            ps = psp.tile([128, 512], F32, tag="ps")
                            rhs = l2dv[:, sb * 32:sb * 32 + 32, 1:9:2, 1:9:2]
                            nc.tensor.matmul(ps[:, 0:512],
                                             wt[:, mh * 128:(mh + 1) * 128],
                                             rhs, start=True, stop=True)
                            bc = COLS['l3b0sc_h0'] if mh == 0 else COLS['l3b0sc_h1']
                            nc.scalar.activation(
                                sc3v[:, mh, sb * 32:sb * 32 + 32, 1:5, 1:5],
                                ps[:, 0:512], AF.Identity, bias=bias_ap(bc),
                                scale=1.0)
                l3sc()

                l3b = act.tile([128, 2 * CC * 36], BF, tag="act")
                l3bv = l3_view(l3b)
                for mh in range(2):
                    v = l3bv[:, mh]
                    nc.gpsimd.memset(v[:, :, 0:1, :], 0.0)
                    nc.gpsimd.memset(v[:, :, 5:6, :], 0.0)
                    nc.gpsimd.memset(v[:, :, 1:5, 0:1], 0.0)
                    nc.gpsimd.memset(v[:, :, 1:5, 5:6], 0.0)
                conv_l3(l3av, l3bv, 'w_l3b0c2',
                        [COLS['l3b0c2_h0'], COLS['l3b0c2_h1']], 2, 1, False,
                        resid_v=sc3v)
                l3c = act.tile([128, 2 * CC * 36], BF, tag="act")
                l3cv = l3_view(l3c)
                for mh in range(2):
                    v = l3cv[:, mh]
                    nc.gpsimd.memset(v[:, :, 0:1, :], 0.0)
                    nc.gpsimd.memset(v[:, :, 5:6, :], 0.0)
                    nc.gpsimd.memset(v[:, :, 1:5, 0:1], 0.0)
                    nc.gpsimd.memset(v[:, :, 1:5, 5:6], 0.0)
                conv_l3(l3bv, l3cv, 'w_l3b1c1',
                        [COLS['l3b1c1_h0'], COLS['l3b1c1_h1']], 2, 1, False)
                l3d = act.tile([128, 2 * CC * 36], BF, tag="act")
                l3dv = l3_view(l3d)
                conv_l3(l3cv, l3dv, 'w_l3b1c2',
                        [COLS['l3b1c2_h0'], COLS['l3b1c2_h1']], 2, 1, False,
                        resid_v=l3bv)

                # ---- head: avgpool + W1 matmul -> oe ----
                featf = sp.tile([128, 2 * CC], F32, tag="feat")
                ffv = featf[:].rearrange("p (h s) -> p h s", h=2)
                nc.vector.tensor_reduce(
                    ffv, l3dv[:, :, :, 1:5, 1:5], mybir.AxisListType.XY, ALU.add)
                feat = sp.tile([128, 2 * CC], BF, tag="featb")
                nc.scalar.activation(feat[:], featf[:], AF.Copy, bias=0.0,
                                     scale=1.0)
                wt = wload('w_head')
                ps = psp.tile([128, 512], F32, tag="ps")
                for h in range(2):
                    nc.tensor.matmul(ps[0:10, 0:CC], wt[:, h * 10:(h + 1) * 10],
                                     feat[:, h * CC:(h + 1) * CC],
                                     start=(h == 0), stop=(h == 1))
                osb = sp.tile([128, CC], F32, tag="osb")
                nc.scalar.activation(osb[0:10, 0:CC], ps[0:10, 0:CC], AF.Copy,
                                     bias=0.0, scale=1.0)
                nc.sync.dma_start(oe[:], osb[0:10, 0:CC])

            # ================= skip network =================
            def skip_net():
                # stem: 7x7 s2 -> 16x16 ch64; patches K=21 (dy,ci), 7 dx taps
                pt, pv4 = new_tight(32, 16, 38)
                xst = xs[:].rearrange("s c y x -> c s y x")
                for g in range(2):
                    for dy in range(7):
                        for ci in range(3):
                            nc.sync.dma_start(
                                pv4[g * 64 + dy * 3 + ci:g * 64 + dy * 3 + ci + 1,
                                    0:32, :, :],
                                xst[ci:ci + 1, g * 32:(g + 1) * 32,
                                    dy:dy + 31:2, 0:38])
                wt = wload('ws_stem')
                s1, s1v = new_act(32, 18)

                def rhs(u, ch, k):
                    if k is None:
                        return list(range(7))
                    g = u
                    s, ih = ch
                    return (pv4[g * 64:g * 64 + 21, 4 * s:4 * s + 4,
                                ih * 8:(ih + 1) * 8, k:k + 31:2],
                            (64 * g, 64 * g))

                def evac(ps, ch):
                    s, ih = ch
                    nc.scalar.activation(
                        s1v[:, 4 * s:4 * s + 4, 1 + ih * 8:1 + (ih + 1) * 8, 1:17],
                        ps[:, 0:512], AF.Relu, bias=bias_ap(COLS['s_stem']),
                        scale=1.0)

                run_conv([(0, 64 * g, 64, g) for g in range(2)], 1,
                         [(s, ih) for s in range(8) for ih in range(2)],
                         rhs, lambda u, k: wt[u * 64:u * 64 + 21,
                                              k * 64:(k + 1) * 64],
                         [(0, evac)])

                # maxpool 3x3 s2 p1: 16x16 -> 8x8
                rm, rmv = new_tight(32, 8, 18)
                nc.vector.tensor_max(rmv[:, :, :, :], s1v[:, :, 0:16:2, :],
                                     s1v[:, :, 1:17:2, :])
                nc.vector.tensor_max(rmv[:, :, :, :], rmv[:, :, :, :],
                                     s1v[:, :, 2:18:2, :])
                s2, s2v = new_act(32, 10)
                nc.vector.tensor_max(s2v[:, :, 1:9, 1:9], rmv[:, :, :, 0:16:2],
                                     rmv[:, :, :, 1:17:2])
                nc.vector.tensor_max(s2v[:, :, 1:9, 1:9], s2v[:, :, 1:9, 1:9],
                                     rmv[:, :, :, 2:18:2])

                # conv2 3x3 s2 64->128: 8x8 -> 4x4; K=64 2 row-tiles
                s3, s3v = new_act(64, 6)
                wt = wload('ws_c2')

                def c2rhs(u, ch, k):
                    if k is None:
                        return list(range(9))
                    g = u
                    (sb,) = ch
                    dy, dx = k // 3, k % 3
                    return (s2v[g * 64:(g + 1) * 64, 0:32, dy:dy + 8:2,
                                dx:dx + 8:2], (64 * g, 0))

                def c2evac_mk(g):
                    def evac(ps, ch):
                        oap = s3v[:, g * 32:(g + 1) * 32, 1:5, 1:5]
                        nc.scalar.activation(oap, ps[:, 0:512], AF.Relu,
                                             bias=bias_ap(COLS['s_c2']),
                                             scale=1.0)
                    return evac

                run_conv([(g, 0, 128, g) for g in range(2)], 2, [(0,)],
                         c2rhs, lambda u, k: wt[u * 64:(u + 1) * 64,
                                                k * 128:(k + 1) * 128],
                         [(0, c2evac_mk(0)), (1, c2evac_mk(1))])

                # conv3 3x3 s2 128->256: 4x4 -> 2x2 (tight out [p,(h,s,2,2)])
                s4 = sp.tile([128, 2 * 64 * 4], BF, tag="s4")
                s4v = s4[:].rearrange("p (h s y x) -> p h s y x", h=2, y=2, x=2)
                wt = wload('ws_c3')
                for mh in range(2):
                    ps = psp.tile([128, 512], F32, tag="ps")
                    for t in range(9):
                        dy, dx = t // 3, t % 3
                        rhs_ap = s3v[:, 0:64, dy:dy + 4:2, dx:dx + 4:2]
                        nc.tensor.matmul(ps[:, 0:256],
                                         wt[:, (mh * 9 + t) * 128:
                                            (mh * 9 + t + 1) * 128],
                                         rhs_ap, start=(t == 0), stop=(t == 8))
                    bc = COLS['s_c3_h0'] if mh == 0 else COLS['s_c3_h1']
                    nc.scalar.activation(s4v[:, mh], ps[:, 0:256], AF.Relu,
                                         bias=bias_ap(bc), scale=1.0)

                # head: avgpool (1/4 folded) + ws_head -> os
                featf = sp.tile([128, 2 * 64], F32, tag="feat")
                ffv = featf[:].rearrange("p (h s) -> p h s", h=2)
                nc.vector.tensor_reduce(ffv, s4v,
                                        mybir.AxisListType.XY, ALU.add)
                feat = sp.tile([128, 2 * 64], BF, tag="featb")
                nc.scalar.activation(feat[:], featf[:], AF.Copy, bias=0.0,
                                     scale=1.0)
                wt = wload('ws_head')
                ps = psp.tile([128, 512], F32, tag="ps")
                for h in range(2):
                    nc.tensor.matmul(ps[0:10, 0:64], wt[:, h * 10:(h + 1) * 10],
                                     feat[:, h * 64:(h + 1) * 64],
                                     start=(h == 0), stop=(h == 1))
                osb = sp.tile([128, 64], F32, tag="osb")
                nc.scalar.activation(osb[0:10, 0:64], ps[0:10, 0:64], AF.Identity,
                                     bias=bias_ap(COLS['s_head'], 0, 10),
                                     scale=1.0)
                nc.sync.dma_start(os_[:], osb[0:10, 0:64])

            def body(_it=0):
                expert_net()
                skip_net()

            if loop_n == 1:
                body()
            else:
                with tc.For_i(0, loop_n, 1) as it:
                    body(it)

    nc.compile()
    return nc


COLS = None  # set by kernel() before build_program


# ---------------------------------------------------------------------------
# Host orchestration
# ---------------------------------------------------------------------------

_CACHE = {}


def _pad_input(x, pad):
    n = x.shape[0]
    out = np.zeros((n, 3, 32 + 2 * pad, 32 + 2 * pad), np.float32)
    out[:, :, pad:pad + 32, pad:pad + 32] = x
    return out.astype(BF_NP)


def _assign_cores(counts):
    """Greedy: cores per expert proportional to counts. Returns list of
    expert-id per core and ccap."""
    k = [1 if c > 0 else 0 for c in counts]
    free = N_CORES - sum(k)
    if free < 0:
        raise RuntimeError("more experts than cores with samples")
    for _ in range(free):
        loads = [counts[e] / k[e] if k[e] else -1 for e in range(E)]
        k[loads.index(max(loads))] += 1
    core_expert = []
    for e in range(E):
        core_expert += [e] * k[e]
    ccap = max(
        (counts[e] + k[e] - 1) // k[e] for e in range(E) if k[e] > 0)
    # round up to multiple of 32 for clean tiling (program assumes /32)
    ccap = max(32, ((ccap + 31) // 32) * 32)
    return core_expert, ccap


def kernel(x, gate_params, expert_params, skip_params, final_W, final_b):
    global COLS
    x = np.asarray(x, np.float32)
    top = _gate_top1(x, gate_params)
    counts = [int((top == e).sum()) for e in range(E)]
    core_expert, ccap = _assign_cores(counts)

    # per-core sample lists (padded with repeats of the first sample)
    order = np.argsort(top, kind='stable')
    by_e = {e: order[top[order] == e] for e in range(E)}
    core_samples = []
    taken = {e: 0 for e in range(E)}
    for e in core_expert:
        idx = by_e[e][taken[e]:taken[e] + ccap]
        taken[e] += len(idx)
        pad = np.full(ccap - len(idx), idx[0] if len(idx) else 0, np.int64)
        core_samples.append(np.concatenate([idx, pad]).astype(np.int64))

    key = ccap
    if key not in _CACHE:
        packs = {}
        for e in sorted(set(core_expert)):
            packs[e], cols = pack_expert(expert_params, e, skip_params,
                                         final_W, final_b)
        COLS = cols
        nc = build_program(ccap)
        _CACHE[key] = (nc, packs, cols)
    nc, packs, COLS = _CACHE[key]

    xp2 = _pad_input(x, 2)
    xp3 = _pad_input(x, 3)
    in_maps = []
    for c in range(N_CORES):
        e = core_expert[c]
        m = dict(packs[e])
        m['xe'] = xp2[core_samples[c]]
        m['xs'] = xp3[c * 64:(c + 1) * 64]
        in_maps.append(m)

    res = bass_utils.run_bass_kernel_spmd(nc, in_maps,
                                          core_ids=list(range(N_CORES)))
    outs = res.results

    y = np.zeros((B, 10), np.float32)
    for c in range(N_CORES):
        y[c * 64:(c + 1) * 64] += np.asarray(outs[c]['os']).T
    # expert contributions: first occurrence wins (padded slots are repeats)
    seen = np.zeros(B, bool)
    for c in range(N_CORES):
        oe_c = np.asarray(outs[c]['oe']).T
        for slot, b in enumerate(core_samples[c]):
            if not seen[b]:
                y[b] += oe_c[slot]
                seen[b] = True
    assert seen.all()
    return y
